# revision 23
# baseline (speedup 1.0000x reference)
import sys
sys.path.insert(0, "/opt/trn_rl_repo")

import numpy as np
from contextlib import ExitStack

from concourse import bass, bacc, tile, mybir
from concourse import bass_utils

F32 = mybir.dt.float32
F32R = mybir.dt.float32r

B = 8
F = 4096
Q = 64
D = 1024
P = 128
PAIRS = 8          # head pairs (16 heads -> 8 pairs of 2)
FC = 256           # feature rows per chunk
NCH = F // FC      # 16 chunks
EPS = 1e-5
SCALE = 64 ** (-0.5)

AF = mybir.ActivationFunctionType


def _bc(ap, p):
    return bass.AP(tensor=ap.tensor, offset=ap.offset, ap=[[0, p]] + list(ap.ap))


def _layernorm_rows(nc, pool_st, x, rows, eps_t, w_bc, b_bc):
    """In-place layernorm over last dim (D=1024) of x[:rows, ...] viewed as [rows, D]."""
    stats = pool_st.tile([rows, 2, 6], F32, tag="stats")
    mv = pool_st.tile([rows, 2], F32, tag="mv")
    sd = pool_st.tile([rows, 1], F32, tag="sd")
    nc.vector.bn_stats(stats[:, 0, :], x[:, 0:512])
    nc.vector.bn_stats(stats[:, 1, :], x[:, 512:1024])
    nc.vector.bn_aggr(mv, stats)
    nc.scalar.activation(sd, mv[:, 1:2], AF.Sqrt, bias=eps_t[:rows], scale=1.0)
    nc.vector.reciprocal(sd, sd)
    nc.vector.tensor_scalar(x, x, mv[:, 0:1], sd,
                            op0=mybir.AluOpType.subtract,
                            op1=mybir.AluOpType.mult)
    nc.vector.tensor_mul(x, x, w_bc[:rows])
    nc.vector.tensor_add(x, x, b_bc[:rows])


def _build_nc():
    nc = bacc.Bacc("TRN2", target_bir_lowering=False, debug=False)

    feat_h = nc.dram_tensor("features", (F, D), F32, kind="ExternalInput")
    lat_h = nc.dram_tensor("latents", (Q, D), F32, kind="ExternalInput")
    lnm_w_h = nc.dram_tensor("lnm_w", (D,), F32, kind="ExternalInput")
    lnm_b_h = nc.dram_tensor("lnm_b", (D,), F32, kind="ExternalInput")
    lnl_w_h = nc.dram_tensor("lnl_w", (D,), F32, kind="ExternalInput")
    lnl_b_h = nc.dram_tensor("lnl_b", (D,), F32, kind="ExternalInput")
    wq_h = nc.dram_tensor("Wq", (D, D), F32, kind="ExternalInput")
    wk_h = nc.dram_tensor("Wk", (D, D), F32, kind="ExternalInput")
    wv_h = nc.dram_tensor("Wv", (D, D), F32, kind="ExternalInput")
    wo_h = nc.dram_tensor("Wo", (D, D), F32, kind="ExternalInput")
    ident_h = nc.dram_tensor("ident", (P, P), F32, kind="ExternalInput")
    fold_h = nc.dram_tensor("fold", (P, Q), F32, kind="ExternalInput")
    out_h = nc.dram_tensor("out", (Q, D), F32, kind="ExternalOutput")

    with tile.TileContext(nc) as tc, ExitStack() as ctx:
        pers = ctx.enter_context(tc.tile_pool(name="pers", bufs=1))
        wstr = ctx.enter_context(tc.tile_pool(name="wstr", bufs=4))
        xp = ctx.enter_context(tc.tile_pool(name="xp", bufs=2))
        stp = ctx.enter_context(tc.tile_pool(name="stp", bufs=4))
        xTp = ctx.enter_context(tc.tile_pool(name="xTp", bufs=2))
        kTp = ctx.enter_context(tc.tile_pool(name="kTp", bufs=2))
        vp = ctx.enter_context(tc.tile_pool(name="vp", bufs=3))
        ptp = ctx.enter_context(tc.tile_pool(name="ptp", bufs=4))

        ps1 = ctx.enter_context(tc.tile_pool(name="ps1", bufs=1, space="PSUM"))
        ps2 = ctx.enter_context(tc.tile_pool(name="ps2", bufs=2, space="PSUM"))
        ps3 = ctx.enter_context(tc.tile_pool(name="ps3", bufs=2, space="PSUM"))
        ps4 = ctx.enter_context(tc.tile_pool(name="ps4", bufs=2, space="PSUM"))
        ps5 = ctx.enter_context(tc.tile_pool(name="ps5", bufs=1, space="PSUM"))

        # ---- persistent SBUF tiles ----
        eps_t = pers.tile([P, 1], F32, tag="eps")
        ident = pers.tile([P, P], F32, tag="ident")
        fold_sb = pers.tile([P, Q], F32, tag="fold")
        lnm_w_bc = pers.tile([P, D], F32, tag="lnmw")
        lnm_b_bc = pers.tile([P, D], F32, tag="lnmb")
        lnl_w_bc = pers.tile([Q, D], F32, tag="lnlw")
        lnl_b_bc = pers.tile([Q, D], F32, tag="lnlb")
        wk = pers.tile([P, 8, D], F32R, tag="wk")
        wv = pers.tile([P, 8, D], F32R, tag="wv")
        qT_pad = pers.tile([P, PAIRS, P], F32, tag="qT")
        out_acc = pers.tile([P, PAIRS, 129], F32, tag="oacc")
        recip_t = pers.tile([P, PAIRS, 1], F32, tag="recip")

        nc.vector.memset(eps_t, EPS)

        # ---- prologue DMAs ----
        nc.gpsimd.dma_start(ident, ident_h[:])
        nc.gpsimd.dma_start(fold_sb, fold_h[:])
        lat_t = xp.tile([Q, D], F32, tag="x_t")
        nc.gpsimd.dma_start(lat_t, lat_h[:])
        nc.gpsimd.dma_start(lnl_w_bc, _bc(lnl_w_h[:], Q))
        nc.gpsimd.dma_start(lnl_b_bc, _bc(lnl_b_h[:], Q))
        nc.gpsimd.dma_start(lnm_w_bc, _bc(lnm_w_h[:], P))
        nc.gpsimd.dma_start(lnm_b_bc, _bc(lnm_b_h[:], P))
        # Wk on default queue (needed early for kT matmuls of chunk 0)
        nc.default_dma_engine.dma_start(
            wk, wk_h[:].rearrange("(c p) d -> p c d", p=P).bitcast(F32R))

        # ---- latents layernorm ----
        _layernorm_rows(nc, stp, lat_t, Q, eps_t, lnl_w_bc, lnl_b_bc)

        # ---- latnT: [P(d-chunk rows), 8, Q] transpose of normalized latents ----
        latnT = kTp.tile([P, 8, Q], F32R, tag="kT_t")
        for g in range(2):
            pst = ps1.tile([P, 4, Q], F32, tag="pst")
            for j in range(4):
                c = g * 4 + j
                nc.tensor.transpose(pst[:, j, :],
                                    lat_t[:, c * P:(c + 1) * P],
                                    ident[0:Q, 0:Q])
            nc.vector.tensor_copy(latnT[:, g * 4:(g + 1) * 4, :], pst)

        # ---- q = latn @ Wq  (streamed Wq chunks) ----
        q_sb = xTp.tile([Q, D], F32, tag="xT_t")
        psq0 = ps3.tile([Q, 512], F32, tag="psv")
        psq1 = ps3.tile([Q, 512], F32, tag="psv")
        psqs = [psq0, psq1]
        for c in range(8):
            wq_t = wstr.tile([P, D], F32R, tag="wchunk")
            nc.gpsimd.dma_start(wq_t, wq_h[c * P:(c + 1) * P, :].bitcast(F32R))
            for h in range(2):
                nc.tensor.matmul(psqs[h],
                                 latnT[:, c, :],
                                 wq_t[:, h * 512:(h + 1) * 512],
                                 start=(c == 0), stop=(c == 7))
        for h in range(2):
            nc.scalar.copy(q_sb[:, h * 512:(h + 1) * 512], psqs[h])

        # Wv after Wq chunks on gpsimd queue
        nc.gpsimd.dma_start(
            wv, wv_h[:].rearrange("(c p) d -> p c d", p=P).bitcast(F32R))

        # ---- qT_pad: block-diagonal padded transpose of q ----
        nc.vector.memset(qT_pad, 0.0)
        for g in range(2):
            psq2 = ps1.tile([P, 4, Q], F32, tag="pst")
            for j in range(4):
                m = g * 4 + j
                nc.tensor.transpose(psq2[:, j, :],
                                    q_sb[:, m * P:(m + 1) * P],
                                    ident[0:Q, 0:Q])
            for j in range(4):
                m = g * 4 + j
                nc.vector.tensor_copy(qT_pad[0:Q, m, 0:Q], psq2[0:Q, j, :])
                nc.vector.tensor_copy(qT_pad[Q:P, m, Q:P], psq2[Q:P, j, :])

        # ---- kT_lat: [P(pair-hd), 8 pairs, Q] = Wk^T-style proj of latents ----
        kT_lat = kTp.tile([P, PAIRS, Q], F32, tag="kT_t")
        for g in range(4):
            psk = ps2.tile([P, 2, Q], F32, tag="psk")
            for j in range(2):
                m = g * 2 + j
                for c in range(8):
                    nc.tensor.matmul(psk[:, j, :],
                                     wk[:, c, m * P:(m + 1) * P],
                                     latnT[:, c, :],
                                     start=(c == 0), stop=(c == 7))
            nc.scalar.copy(kT_lat[:, g * 2:(g + 1) * 2, :], psk)

        # ---- v_lat: [Q, 8 pairs, 129] with ones in col 128 ----
        v_lat = vp.tile([Q, PAIRS, 129], F32, tag="v_t")
        nc.vector.memset(v_lat[:, :, 128:129], 1.0)
        for h in range(2):
            psv = ps3.tile([Q, 512], F32, tag="psv")
            for c in range(8):
                nc.tensor.matmul(psv,
                                 latnT[:, c, :],
                                 wv[:, c, h * 512:(h + 1) * 512],
                                 start=(c == 0), stop=(c == 7))
            nc.scalar.copy(v_lat[:, h * 4:(h + 1) * 4, 0:128],
                           psv.rearrange("q (m x) -> q m x", m=4))

        # ---- PT_lat + out_acc init ----
        for mm in range(0, PAIRS, 2):
            pso = ps5.tile([P, 2, 129], F32, tag="pso")
            pspt = ps4.tile([P, 4, P], F32, tag="pspt")
            for j in range(2):
                m = mm + j
                nc.tensor.matmul(pspt[0:Q, j, :],
                                 kT_lat[:, m, :],
                                 qT_pad[:, m, :],
                                 start=True, stop=True)
                pt_sb = ptp.tile([P, P], F32, tag="pt_sb")
                nc.scalar.activation(pt_sb[0:Q, :], pspt[0:Q, j, :],
                                     AF.Exp, bias=0.0, scale=SCALE)
                nc.tensor.matmul(pso[:, j, :],
                                 pt_sb[0:Q, :],
                                 v_lat[:, m, :],
                                 start=True, stop=True)
            nc.vector.tensor_copy(out_acc[:, mm:mm + 2, :], pso)

        # ==== main loop over 16 feature chunks ====
        for ci in range(NCH):
            x_t = xp.tile([P, 2, D], F32, tag="x_t")
            nc.default_dma_engine.dma_start(
                x_t,
                feat_h[ci * FC:(ci + 1) * FC, :].rearrange(
                    "(i p) d -> p i d", p=P))
            for i in range(2):
                _layernorm_rows(nc, stp, x_t[:, i, :], P, eps_t,
                                lnm_w_bc, lnm_b_bc)

            # xT_t: [P(d rows), 8 c, FC]
            xT_t = xTp.tile([P, 8, FC], F32R, tag="xT_t")
            for g in range(4):
                pst = ps1.tile([P, 2, FC], F32, tag="pst")
                for c2 in range(2):
                    c = g * 2 + c2
                    for i in range(2):
                        nc.tensor.transpose(pst[:, c2, i * P:(i + 1) * P],
                                            x_t[:, i, c * P:(c + 1) * P],
                                            ident)
                nc.vector.tensor_copy(xT_t[:, g * 2:(g + 1) * 2, :], pst)

            # kT_t: [P(pair-hd), 8 pairs, FC]
            kT_t = kTp.tile([P, PAIRS, FC], F32, tag="kT_t")
            for g in range(4):
                psk = ps2.tile([P, 2, FC], F32, tag="psk")
                for j in range(2):
                    m = g * 2 + j
                    for c in range(8):
                        nc.tensor.matmul(
                            psk[:, j, :],
                            wk[:, c, m * P:(m + 1) * P],
                            xT_t[:, c, :],
                            start=(c == 0), stop=(c == 7))
                nc.scalar.copy(kT_t[:, g * 2:(g + 1) * 2, :], psk)

            # v: two [P, 8 pairs, 129] tiles (fs=0: rows 0..127, fs=1: 128..255)
            v_ts = []
            for fs in range(2):
                vt = vp.tile([P, PAIRS, 129], F32, tag="v_t")
                nc.vector.memset(vt[:, :, 128:129], 1.0)
                v_ts.append(vt)
            for fs in range(2):
                for h in range(2):
                    psv = ps3.tile([P, 512], F32, tag="psv")
                    for c in range(8):
                        nc.tensor.matmul(
                            psv,
                            xT_t[:, c, fs * P:(fs + 1) * P],
                            wv[:, c, h * 512:(h + 1) * 512],
                            start=(c == 0), stop=(c == 7))
                    nc.scalar.copy(v_ts[fs][:, h * 4:(h + 1) * 4, 0:128],
                                   psv.rearrange("p (m x) -> p m x", m=4))

            # PT + out accumulate
            for mm in range(0, PAIRS, 2):
                pso = ps5.tile([P, 2, 129], F32, tag="pso")
                pspt = ps4.tile([P, 4, P], F32, tag="pspt")
                for j in range(2):
                    m = mm + j
                    for fs in range(2):
                        nc.tensor.matmul(
                            pspt[:, 2 * j + fs, :],
                            kT_t[:, m, fs * P:(fs + 1) * P],
                            qT_pad[:, m, :],
                            start=True, stop=True)
                        pt_sb = ptp.tile([P, P], F32, tag="pt_sb")
                        nc.scalar.activation(pt_sb, pspt[:, 2 * j + fs, :],
                                             AF.Exp, bias=0.0, scale=SCALE)
                        nc.tensor.matmul(
                            pso[:, j, :],
                            pt_sb,
                            v_ts[fs][:, m, :],
                            start=(fs == 0), stop=(fs == 1))
                nc.vector.tensor_add(out_acc[:, mm:mm + 2, :],
                                     out_acc[:, mm:mm + 2, :], pso)

        # ==== epilogue ====
        nc.vector.reciprocal(recip_t, out_acc[:, :, 128:129])
        norm2 = vp.tile([P, PAIRS, P], F32, tag="v_t")
        nc.vector.memset(norm2, 0.0)
        for m in range(PAIRS):
            nc.vector.tensor_scalar_mul(norm2[0:Q, m, 0:Q],
                                        out_acc[0:Q, m, 0:Q],
                                        recip_t[0:Q, m, :])
            nc.vector.tensor_scalar_mul(norm2[Q:P, m, Q:P],
                                        out_acc[Q:P, m, Q:P],
                                        recip_t[Q:P, m, :])

        attnT = kTp.tile([P, PAIRS, P], F32R, tag="kT_t")
        for g in range(2):
            psa = ps4.tile([P, 4, P], F32, tag="pspt")
            for j in range(4):
                m = g * 4 + j
                nc.tensor.transpose(psa[:, j, :], norm2[:, m, :], ident)
            nc.vector.tensor_copy(attnT[:, g * 4:(g + 1) * 4, :], psa)

        foldin = xTp.tile([P, D], F32, tag="xT_t")
        psO0 = ps3.tile([P, 512], F32, tag="psv")
        psO1 = ps3.tile([P, 512], F32, tag="psv")
        psOs = [psO0, psO1]
        for m in range(PAIRS):
            wo_t = wstr.tile([P, D], F32R, tag="wchunk")
            nc.default_dma_engine.dma_start(
                wo_t, wo_h[m * P:(m + 1) * P, :].bitcast(F32R))
            for h in range(2):
                nc.tensor.matmul(psOs[h],
                                 attnT[:, m, :],
                                 wo_t[:, h * 512:(h + 1) * 512],
                                 start=(m == 0), stop=(m == 7))
        for h in range(2):
            nc.scalar.copy(foldin[:, h * 512:(h + 1) * 512], psOs[h])

        out_sb = xp.tile([Q, D], F32, tag="x_t")
        for h in range(2):
            psF = ps2.tile([Q, 512], F32, tag="psk")
            nc.tensor.matmul(psF, fold_sb,
                             foldin[:, h * 512:(h + 1) * 512],
                             start=True, stop=True)
            nc.scalar.copy(out_sb[:, h * 512:(h + 1) * 512], psF)
        nc.default_dma_engine.dma_start(out_h[:], out_sb)

    return nc


_nc_cache = None


def _get_nc():
    global _nc_cache
    if _nc_cache is None:
        _nc_cache = _build_nc()
    return _nc_cache


def _in_maps(inputs):
    shared = {}
    for k in ("lnm_w", "lnm_b", "lnl_w", "lnl_b", "Wq", "Wk", "Wv", "Wo"):
        shared[k] = np.ascontiguousarray(np.asarray(inputs[k], np.float32))
    shared["ident"] = np.eye(P, dtype=np.float32)
    shared["fold"] = np.concatenate(
        [np.eye(Q, dtype=np.float32)] * 2, axis=0)
    feats = np.asarray(inputs["features"], np.float32)
    lats = np.asarray(inputs["latents"], np.float32)
    maps = []
    for b in range(B):
        m = dict(shared)
        m["features"] = np.ascontiguousarray(feats[b])
        m["latents"] = np.ascontiguousarray(lats[b])
        maps.append(m)
    return maps


def run(inputs, trace=False, tmpdir=None):
    nc = _get_nc()
    if not nc.is_finalized():
        nc.finalize()
    maps = _in_maps(inputs)
    res = bass_utils.run_bass_kernel_spmd(
        nc, maps, list(range(B)), tmpdir=tmpdir, trace=trace)
    out = np.stack([res.results[b]["out"] for b in range(B)], axis=0)
    return out.astype(np.float32), res


def kernel(**inputs):
    out, _ = run(inputs)
    return out
